# revision 10
# baseline (speedup 1.0000x reference)
"""Cross-attention kernel for Trainium2, data-parallel over batch on 8 NeuronCores.

Per core (batch element b):
  q = x[b] @ Wq.T + bq ; k = c[b] @ Wk.T + bk ; v = c[b] @ Wv.T + bv
  out[b] = softmax(q @ k.T / sqrt(D)) @ v

Algebraic fold (host side): scores = q k^T = x (Wq^T Wk) c^T + rank-1 terms.
With M = Wq^T Wk and t = x M + bq Wk, scores = t c^T + f(s) where the f(s)
term is constant along the key axis and cancels in softmax exactly. This
removes the K projection entirely (256 of 1792 matmuls) and lets phase 2
stream c^T straight from SBUF as the key operand.

Device layout (all matmul operands bf16, fp32 accumulation):
  phase 1: TT[d',s] = (M^T-free: stationary M[d,d'] tiles) x^T, scaled 1/sqrt(D)
           V[t,e]  = c @ Wv.T + bv                                (t on partitions)
  phase 2: per 128-row tile of s: S = TT.T @ CT (psum), P = exp(S) + row sums
           via ACT, P transposed 128x128 via DMA xbar, O = P @ V, scaled by
           1/rowsum on drain.
"""

import numpy as np
import ml_dtypes

import concourse.bass as bass
import concourse.mybir as mybir
import concourse.tile as tile
from concourse import bacc
from concourse.bass_utils import run_bass_kernel_spmd

DIM = 1024
SEQ = 2048
B = 8
P = 128
DT = DIM // P        # 8 contraction tiles of 128
ST = SEQ // P        # 16 seq tiles of 128
KC = SEQ // 512      # 4 key chunks of 512
EC = DIM // 512      # 2 embed chunks of 512
F32 = mybir.dt.float32
BF16 = mybir.dt.bfloat16

_CACHED_NC = None


def build_nc():
    nc = bacc.Bacc(None, target_bir_lowering=False)

    xt = nc.declare_dram_parameter("xt", [DIM, SEQ], BF16, isOutput=False)
    ct = nc.declare_dram_parameter("ct", [DIM, SEQ], BF16, isOutput=False)
    # m pre-tiled on host: m[et, p, dt, j] = M[dt*128+p, et*128+j], so each
    # et tile lands as one DMA with 2 KB-contiguous per-partition rows.
    m = nc.declare_dram_parameter("m", [DT, P, DT, P], BF16, isOutput=False)
    # wv pre-tiled on host: wv[p, dt, e] = Wv[e, dt*128+p] (8 KB/partition).
    wvt = nc.declare_dram_parameter("wvt", [P, DT, DIM], BF16, isOutput=False)
    bts = nc.declare_dram_parameter("bts", [DT, P], F32, isOutput=False)
    bvb = nc.declare_dram_parameter("bvb", [P, DIM], F32, isOutput=False)
    out = nc.declare_dram_parameter("out", [SEQ, DIM], F32, isOutput=True)

    xt_r = xt.rearrange("(t p) s -> p t s", p=P)
    ct_r = ct.rearrange("(t p) s -> p t s", p=P)
    m_r = m.rearrange("e p t j -> p e t j")
    out_r = out.rearrange("(t p) e -> p t e", p=P)

    with tile.TileContext(nc) as tc:
        with (
            tc.tile_pool(name="resid", bufs=1) as resid,
            tc.tile_pool(name="singles", bufs=1) as singles,
        ):
            tt_sb = resid.tile([P, DT, SEQ], BF16, tag="tt")
            ct_sb = resid.tile([P, DT, SEQ], BF16, tag="ct")
            v_sb = resid.tile([P, ST, DIM], BF16, tag="v")

            bt_sb = singles.tile([P, DT], F32, tag="bt")
            bv_sb = singles.tile([P, DIM], F32, tag="bv")

            # ---------------- phase 1: projections ----------------
            with (
                tc.tile_pool(name="acts", bufs=1) as acts,
                tc.tile_pool(name="mpool", bufs=1) as mpool,
                tc.tile_pool(name="wvpool", bufs=1) as wvpool,
                tc.tile_pool(name="warmps", bufs=1, space="PSUM") as warmps,
                tc.tile_pool(name="ppool", bufs=6, space="PSUM") as ppool,
            ):
                # Dummy matmuls on a zeroed tile keep the PE busy through the
                # input-DMA window: HAM un-throttles before the real matmuls
                # start, instead of ramping on them.
                wsrc = acts.tile([P, 512], BF16, tag="warm")
                nc.vector.memset(wsrc, 0.0)
                wps = warmps.tile([P, 512], F32, tag="wps")
                for i in range(20):
                    nc.tensor.matmul(
                        wps, wsrc[:, 0:P], wsrc, start=(i == 0), stop=(i == 19)
                    )
                xt_sb = acts.tile([P, DT, SEQ], BF16, tag="xt")
                # et-major so each et tile is one contiguous 2 KB/partition DMA
                m_sb = mpool.tile([P, DT, DT, P], BF16, tag="m")
                wv_t = wvpool.tile([P, DT, DIM], BF16, tag="wv")
                # Input DMA schedule, interleaved across the two HWDGE engines
                # (cheap descriptor issue) in consumption order, with gpsimd's
                # SWDGE path (~1us descriptor generation per dma_start)
                # carrying a few big batched transfers needed later.
                # Per-ring DMA throughput is descriptor-count-bound
                # (~18ns/desc), so every transfer here uses >=2KB
                # per-partition descriptors: xt in 1024-col half-row chunks
                # (the sc0+sc1 passes share half 0), m0/m1 as 2KB-desc tiles
                # on the HWDGE rings, and everything else as big batched
                # gpsimd transfers with 4-16KB descriptors.
                def xt_load(eng, h, dts):
                    for dt in dts:
                        eng.dma_start(
                            out=xt_sb[:, dt, h * 1024 : (h + 1) * 1024],
                            in_=xt_r[:, dt, h * 1024 : (h + 1) * 1024],
                        )

                nc.sync.dma_start(out=m_sb[:, 0], in_=m_r[:, 0])
                nc.scalar.dma_start(out=m_sb[:, 1], in_=m_r[:, 1])
                xt_load(nc.sync, 0, (0, 1, 2))
                xt_load(nc.scalar, 0, (4, 5, 6))
                xt_load(nc.gpsimd, 0, (3, 7))
                xt_load(nc.sync, 1, range(0, 4))
                xt_load(nc.scalar, 1, range(4, 8))
                nc.gpsimd.dma_start(out=bt_sb, in_=bts.rearrange("t p -> p t"))
                nc.gpsimd.dma_start(out=m_sb[:, 2:8], in_=m_r[:, 2:8])
                nc.gpsimd.dma_start(out=bv_sb, in_=bvb[:, :])
                nc.gpsimd.dma_start(out=ct_sb[:, :], in_=ct_r[:, :, :])
                nc.gpsimd.dma_start(out=wv_t[:, :, :], in_=wvt[:, :, :])

                # t projection: out[d'128, s512] accumulated over d.
                # sc-major so each 1 MB xt chunk feeds 8 full groups.
                for sc in range(KC):
                    for et in range(DT):
                        ps = ppool.tile([P, 512], F32, tag="proj")
                        for dt in range(DT):
                            nc.tensor.matmul(
                                ps,
                                m_sb[:, et, dt],
                                xt_sb[:, dt, sc * 512 : (sc + 1) * 512],
                                start=(dt == 0),
                                stop=(dt == DT - 1),
                            )
                        nc.scalar.activation(
                            out=tt_sb[:, et, sc * 512 : (sc + 1) * 512],
                            in_=ps,
                            func=mybir.ActivationFunctionType.Identity,
                            bias=bt_sb[:, et : et + 1],
                            scale=1.0 / 32.0,
                        )

                # v projection: out[t128, e512], CT tiles stationary
                for tt in range(ST):
                    for ec in range(EC):
                        ps = ppool.tile([P, 512], F32, tag="proj")
                        for dt in range(DT):
                            nc.tensor.matmul(
                                ps,
                                ct_sb[:, dt, tt * P : (tt + 1) * P],
                                wv_t[:, dt, ec * 512 : (ec + 1) * 512],
                                start=(dt == 0),
                                stop=(dt == DT - 1),
                            )
                        nc.vector.tensor_add(
                            out=v_sb[:, tt, ec * 512 : (ec + 1) * 512],
                            in0=ps,
                            in1=bv_sb[:, ec * 512 : (ec + 1) * 512],
                        )

            # ---------------- phase 2: attention ----------------
            # Software-pipelined: S/exp/transpose for tile st is emitted
            # before O/store for tile st-1, so the PE chews on S(st) while
            # the xbar transpose of P(st-1) completes.
            with (
                tc.tile_pool(name="attn", bufs=3) as attn,
                tc.tile_pool(name="stats", bufs=4) as stats,
                tc.tile_pool(name="spsum", bufs=5, space="PSUM") as spsum,
                tc.tile_pool(name="opsum", bufs=3, space="PSUM") as opsum,
            ):
                def emit_s_stage(st):
                    p_sb = attn.tile([P, SEQ], BF16, tag="p")
                    sums = stats.tile([P, KC], F32, tag="sums")
                    for kc in range(KC):
                        sp = spsum.tile([P, 512], F32, tag="s")
                        for dt in range(DT):
                            nc.tensor.matmul(
                                sp,
                                tt_sb[:, dt, st * P : (st + 1) * P],
                                ct_sb[:, dt, kc * 512 : (kc + 1) * 512],
                                start=(dt == 0),
                                stop=(dt == DT - 1),
                            )
                        nc.scalar.activation(
                            out=p_sb[:, kc * 512 : (kc + 1) * 512],
                            in_=sp,
                            func=mybir.ActivationFunctionType.Exp,
                            accum_out=sums[:, kc : kc + 1],
                        )
                    ssum = stats.tile([P, 1], F32, tag="ssum")
                    rsum = stats.tile([P, 1], F32, tag="rsum")
                    nc.vector.reduce_sum(out=ssum, in_=sums, axis=mybir.AxisListType.X)
                    nc.vector.reciprocal(out=rsum, in_=ssum)

                    # One xbar transpose for the whole row block:
                    # pt[p, tt, f] = p_sb[f, tt*128 + p]
                    pt_sb = attn.tile([P, ST, P], BF16, tag="pt")
                    nc.sync.dma_start_transpose(out=pt_sb, in_=p_sb[:, :])
                    return pt_sb, rsum

                def emit_o_stage(st, pt_sb, rsum):
                    for ec in range(EC):
                        op = opsum.tile([P, 512], F32, tag="o")
                        for tt in range(ST):
                            nc.tensor.matmul(
                                op,
                                pt_sb[:, tt],
                                v_sb[:, tt, ec * 512 : (ec + 1) * 512],
                                start=(tt == 0),
                                stop=(tt == ST - 1),
                            )
                        o_sb = attn.tile([P, 512], F32, tag="o")
                        nc.vector.tensor_scalar_mul(
                            out=o_sb, in0=op, scalar1=rsum
                        )
                        nc.scalar.dma_start(
                            out=out_r[:, st, ec * 512 : (ec + 1) * 512], in_=o_sb
                        )

                pending = None
                for st in range(ST):
                    cur = emit_s_stage(st)
                    if pending is not None:
                        emit_o_stage(st - 1, *pending)
                    pending = cur
                emit_o_stage(ST - 1, *pending)

    nc.compile()
    return nc


def prep_inputs(x, context, Wq, bq, Wk, bk, Wv, bv):
    """Host-side prep: per-batch transposed bf16 activations, the folded
    score matrix M = Wq^T Wk (bf16), transposed bf16 V weight, tiled fp32
    biases. Returns per-core input maps."""
    bf = ml_dtypes.bfloat16
    Wq = np.asarray(Wq, dtype=np.float32)
    Wk = np.asarray(Wk, dtype=np.float32)
    mfull = Wq.T @ Wk
    # m[et, p, dt, j] = M[dt*128+p, et*128+j]
    m = np.ascontiguousarray(
        mfull.reshape(DT, P, DT, P).transpose(2, 1, 0, 3)
    ).astype(bf)
    # wvt[p, dt, e] = Wv.T[dt*128+p, e]
    wvt = np.ascontiguousarray(
        np.asarray(Wv, dtype=np.float32).T.reshape(DT, P, DIM).transpose(1, 0, 2)
    ).astype(bf)
    # t = x M + bq Wk; the q.bk rank-1 term is constant along keys and
    # cancels in softmax. 1/sqrt(D) folded into the t projection.
    bt = (np.asarray(bq, dtype=np.float32) @ Wk) / 32.0
    bts = bt.reshape(DT, P)
    bvb = np.ascontiguousarray(
        np.broadcast_to(np.asarray(bv, dtype=np.float32), (P, DIM))
    )
    in_maps = []
    for b in range(B):
        in_maps.append(
            {
                "xt": np.ascontiguousarray(x[b].T).astype(bf),
                "ct": np.ascontiguousarray(context[b].T).astype(bf),
                "m": m,
                "wvt": wvt,
                "bts": bts,
                "bvb": bvb,
            }
        )
    return in_maps


def kernel(x, context, Wq, bq, Wk, bk, Wv, bv):
    global _CACHED_NC
    x = np.asarray(x, dtype=np.float32)
    context = np.asarray(context, dtype=np.float32)
    in_maps = prep_inputs(x, context, Wq, bq, Wk, bk, Wv, bv)
    if _CACHED_NC is None:
        _CACHED_NC = build_nc()
    nc = _CACHED_NC
    core_ids = list(range(B))
    res = run_bass_kernel_spmd(nc, in_maps, core_ids)
    return np.stack([res.results[i]["out"] for i in core_ids]).astype(np.float32)


# revision 12
# speedup vs baseline: 1.0007x; 1.0007x over previous
"""Cross-attention kernel for Trainium2, data-parallel over batch on 8 NeuronCores.

Per core (batch element b):
  q = x[b] @ Wq.T + bq ; k = c[b] @ Wk.T + bk ; v = c[b] @ Wv.T + bv
  out[b] = softmax(q @ k.T / sqrt(D)) @ v

Algebraic fold (host side): scores = q k^T = x (Wq^T Wk) c^T + rank-1 terms.
With M = Wq^T Wk and t = x M + bq Wk, scores = t c^T + f(s) where the f(s)
term is constant along the key axis and cancels in softmax exactly. This
removes the K projection entirely (256 of 1792 matmuls) and lets phase 2
stream c^T straight from SBUF as the key operand.

Device layout (all matmul operands bf16, fp32 accumulation):
  phase 1: TT[d',s] = (M^T-free: stationary M[d,d'] tiles) x^T, scaled 1/sqrt(D)
           V[t,e]  = c @ Wv.T + bv                                (t on partitions)
  phase 2: per 128-row tile of s: S = TT.T @ CT (psum), P = exp(S) + row sums
           via ACT, P transposed 128x128 via DMA xbar, O = P @ V, scaled by
           1/rowsum on drain.
"""

import numpy as np
import ml_dtypes

import concourse.bass as bass
import concourse.mybir as mybir
import concourse.tile as tile
from concourse import bacc
from concourse.bass_utils import run_bass_kernel_spmd

DIM = 1024
SEQ = 2048
B = 8
P = 128
DT = DIM // P        # 8 contraction tiles of 128
ST = SEQ // P        # 16 seq tiles of 128
KC = SEQ // 512      # 4 key chunks of 512
EC = DIM // 512      # 2 embed chunks of 512
F32 = mybir.dt.float32
BF16 = mybir.dt.bfloat16

_CACHED_NC = None


def build_nc():
    nc = bacc.Bacc(None, target_bir_lowering=False)

    xt = nc.declare_dram_parameter("xt", [DIM, SEQ], BF16, isOutput=False)
    ct = nc.declare_dram_parameter("ct", [DIM, SEQ], BF16, isOutput=False)
    # m pre-tiled on host: m[et, p, dt, j] = M[dt*128+p, et*128+j], so each
    # et tile lands as one DMA with 2 KB-contiguous per-partition rows.
    m = nc.declare_dram_parameter("m", [DT, P, DT, P], BF16, isOutput=False)
    # wv pre-tiled on host: wv[p, dt, e] = Wv[e, dt*128+p] (8 KB/partition).
    wvt = nc.declare_dram_parameter("wvt", [P, DT, DIM], BF16, isOutput=False)
    bts = nc.declare_dram_parameter("bts", [DT, P], F32, isOutput=False)
    bvb = nc.declare_dram_parameter("bvb", [P, DIM], F32, isOutput=False)
    out = nc.declare_dram_parameter("out", [SEQ, DIM], F32, isOutput=True)

    xt_r = xt.rearrange("(t p) s -> p t s", p=P)
    ct_r = ct.rearrange("(t p) s -> p t s", p=P)
    m_r = m.rearrange("e p t j -> p e t j")
    out_r = out.rearrange("(t p) e -> p t e", p=P)

    with tile.TileContext(nc) as tc:
        with (
            tc.tile_pool(name="resid", bufs=1) as resid,
            tc.tile_pool(name="singles", bufs=1) as singles,
        ):
            tt_sb = resid.tile([P, DT, SEQ], BF16, tag="tt")
            ct_sb = resid.tile([P, DT, SEQ], BF16, tag="ct")
            v_sb = resid.tile([P, ST, DIM], BF16, tag="v")

            bt_sb = singles.tile([P, DT], F32, tag="bt")
            bv_sb = singles.tile([P, DIM], F32, tag="bv")

            # ---------------- phase 1: projections ----------------
            with (
                tc.tile_pool(name="acts", bufs=1) as acts,
                tc.tile_pool(name="mpool", bufs=1) as mpool,
                tc.tile_pool(name="wvpool", bufs=1) as wvpool,
                tc.tile_pool(name="warmps", bufs=1, space="PSUM") as warmps,
                tc.tile_pool(name="ppool", bufs=6, space="PSUM") as ppool,
            ):
                # Dummy matmuls on a zeroed tile keep the PE busy through the
                # input-DMA window: HAM un-throttles before the real matmuls
                # start, instead of ramping on them.
                wsrc = acts.tile([P, 512], BF16, tag="warm")
                nc.vector.memset(wsrc, 0.0)
                wps = warmps.tile([P, 512], F32, tag="wps")
                for i in range(20):
                    nc.tensor.matmul(
                        wps, wsrc[:, 0:P], wsrc, start=(i == 0), stop=(i == 19)
                    )
                xt_sb = acts.tile([P, DT, SEQ], BF16, tag="xt")
                # et-major so each et tile is one contiguous 2 KB/partition DMA
                m_sb = mpool.tile([P, DT, DT, P], BF16, tag="m")
                wv_t = wvpool.tile([P, DT, DIM], BF16, tag="wv")
                # Input DMA schedule, interleaved across the two HWDGE engines
                # (cheap descriptor issue) in consumption order, with gpsimd's
                # SWDGE path (~1us descriptor generation per dma_start)
                # carrying a few big batched transfers needed later.
                # Per-ring DMA throughput is descriptor-count-bound
                # (~18ns/desc), so every transfer here uses >=2KB
                # per-partition descriptors: xt in 1024-col half-row chunks
                # (the sc0+sc1 passes share half 0), m0/m1 as 2KB-desc tiles
                # on the HWDGE rings, and everything else as big batched
                # gpsimd transfers with 4-16KB descriptors.
                def xt_load(eng, h, dts):
                    for dt in dts:
                        eng.dma_start(
                            out=xt_sb[:, dt, h * 1024 : (h + 1) * 1024],
                            in_=xt_r[:, dt, h * 1024 : (h + 1) * 1024],
                        )

                def m_load(eng, et):
                    eng.dma_start(out=m_sb[:, et], in_=m_r[:, et])

                m_load(nc.sync, 0)
                m_load(nc.scalar, 1)
                xt_load(nc.sync, 0, (0, 1, 2))
                xt_load(nc.scalar, 0, (4, 5, 6))
                xt_load(nc.gpsimd, 0, (3, 7))
                for et in range(2, DT):
                    m_load(nc.sync if et % 2 == 0 else nc.scalar, et)
                xt_load(nc.sync, 1, range(0, 4))
                xt_load(nc.scalar, 1, range(4, 8))
                nc.gpsimd.dma_start(out=bt_sb, in_=bts.rearrange("t p -> p t"))
                nc.gpsimd.dma_start(out=bv_sb, in_=bvb[:, :])
                nc.gpsimd.dma_start(out=ct_sb[:, :], in_=ct_r[:, :, :])
                nc.gpsimd.dma_start(out=wv_t[:, :, :], in_=wvt[:, :, :])

                # t projection: out[d'128, s512] accumulated over d.
                # First two sc passes (both inside xt half 0) are interleaved
                # per et so each m tile is consumed at the rate the DMA rings
                # deliver them; the sc2/sc3 passes then run with everything
                # resident.
                def t_group(sc, et):
                    ps = ppool.tile([P, 512], F32, tag="proj")
                    for dt in range(DT):
                        nc.tensor.matmul(
                            ps,
                            m_sb[:, et, dt],
                            xt_sb[:, dt, sc * 512 : (sc + 1) * 512],
                            start=(dt == 0),
                            stop=(dt == DT - 1),
                        )
                    nc.scalar.activation(
                        out=tt_sb[:, et, sc * 512 : (sc + 1) * 512],
                        in_=ps,
                        func=mybir.ActivationFunctionType.Identity,
                        bias=bt_sb[:, et : et + 1],
                        scale=1.0 / 32.0,
                    )

                for et in range(DT):
                    t_group(0, et)
                    t_group(1, et)
                for sc in (2, 3):
                    for et in range(DT):
                        t_group(sc, et)

                # v projection: out[t128, e512], CT tiles stationary
                for tt in range(ST):
                    for ec in range(EC):
                        ps = ppool.tile([P, 512], F32, tag="proj")
                        for dt in range(DT):
                            nc.tensor.matmul(
                                ps,
                                ct_sb[:, dt, tt * P : (tt + 1) * P],
                                wv_t[:, dt, ec * 512 : (ec + 1) * 512],
                                start=(dt == 0),
                                stop=(dt == DT - 1),
                            )
                        nc.vector.tensor_add(
                            out=v_sb[:, tt, ec * 512 : (ec + 1) * 512],
                            in0=ps,
                            in1=bv_sb[:, ec * 512 : (ec + 1) * 512],
                        )

            # ---------------- phase 2: attention ----------------
            # Software-pipelined: S/exp/transpose for tile st is emitted
            # before O/store for tile st-1, so the PE chews on S(st) while
            # the xbar transpose of P(st-1) completes.
            with (
                tc.tile_pool(name="attn", bufs=3) as attn,
                tc.tile_pool(name="stats", bufs=4) as stats,
                tc.tile_pool(name="spsum", bufs=5, space="PSUM") as spsum,
                tc.tile_pool(name="opsum", bufs=3, space="PSUM") as opsum,
            ):
                def emit_s_stage(st):
                    p_sb = attn.tile([P, SEQ], BF16, tag="p")
                    sums = stats.tile([P, KC], F32, tag="sums")
                    for kc in range(KC):
                        sp = spsum.tile([P, 512], F32, tag="s")
                        for dt in range(DT):
                            nc.tensor.matmul(
                                sp,
                                tt_sb[:, dt, st * P : (st + 1) * P],
                                ct_sb[:, dt, kc * 512 : (kc + 1) * 512],
                                start=(dt == 0),
                                stop=(dt == DT - 1),
                            )
                        nc.scalar.activation(
                            out=p_sb[:, kc * 512 : (kc + 1) * 512],
                            in_=sp,
                            func=mybir.ActivationFunctionType.Exp,
                            accum_out=sums[:, kc : kc + 1],
                        )
                    ssum = stats.tile([P, 1], F32, tag="ssum")
                    rsum = stats.tile([P, 1], F32, tag="rsum")
                    nc.vector.reduce_sum(out=ssum, in_=sums, axis=mybir.AxisListType.X)
                    nc.vector.reciprocal(out=rsum, in_=ssum)

                    # One xbar transpose for the whole row block:
                    # pt[p, tt, f] = p_sb[f, tt*128 + p]
                    pt_sb = attn.tile([P, ST, P], BF16, tag="pt")
                    nc.sync.dma_start_transpose(out=pt_sb, in_=p_sb[:, :])
                    return pt_sb, rsum

                def emit_o_stage(st, pt_sb, rsum):
                    for ec in range(EC):
                        op = opsum.tile([P, 512], F32, tag="o")
                        for tt in range(ST):
                            nc.tensor.matmul(
                                op,
                                pt_sb[:, tt],
                                v_sb[:, tt, ec * 512 : (ec + 1) * 512],
                                start=(tt == 0),
                                stop=(tt == ST - 1),
                            )
                        o_sb = attn.tile([P, 512], F32, tag="o")
                        nc.vector.tensor_scalar_mul(
                            out=o_sb, in0=op, scalar1=rsum
                        )
                        nc.scalar.dma_start(
                            out=out_r[:, st, ec * 512 : (ec + 1) * 512], in_=o_sb
                        )

                pending = None
                for st in range(ST):
                    cur = emit_s_stage(st)
                    if pending is not None:
                        emit_o_stage(st - 1, *pending)
                    pending = cur
                emit_o_stage(ST - 1, *pending)

    nc.compile()
    return nc


def prep_inputs(x, context, Wq, bq, Wk, bk, Wv, bv):
    """Host-side prep: per-batch transposed bf16 activations, the folded
    score matrix M = Wq^T Wk (bf16), transposed bf16 V weight, tiled fp32
    biases. Returns per-core input maps."""
    bf = ml_dtypes.bfloat16
    Wq = np.asarray(Wq, dtype=np.float32)
    Wk = np.asarray(Wk, dtype=np.float32)
    mfull = Wq.T @ Wk
    # m[et, p, dt, j] = M[dt*128+p, et*128+j]
    m = np.ascontiguousarray(
        mfull.reshape(DT, P, DT, P).transpose(2, 1, 0, 3)
    ).astype(bf)
    # wvt[p, dt, e] = Wv.T[dt*128+p, e]
    wvt = np.ascontiguousarray(
        np.asarray(Wv, dtype=np.float32).T.reshape(DT, P, DIM).transpose(1, 0, 2)
    ).astype(bf)
    # t = x M + bq Wk; the q.bk rank-1 term is constant along keys and
    # cancels in softmax. 1/sqrt(D) folded into the t projection.
    bt = (np.asarray(bq, dtype=np.float32) @ Wk) / 32.0
    bts = bt.reshape(DT, P)
    bvb = np.ascontiguousarray(
        np.broadcast_to(np.asarray(bv, dtype=np.float32), (P, DIM))
    )
    in_maps = []
    for b in range(B):
        in_maps.append(
            {
                "xt": np.ascontiguousarray(x[b].T).astype(bf),
                "ct": np.ascontiguousarray(context[b].T).astype(bf),
                "m": m,
                "wvt": wvt,
                "bts": bts,
                "bvb": bvb,
            }
        )
    return in_maps


def kernel(x, context, Wq, bq, Wk, bk, Wv, bv):
    global _CACHED_NC
    x = np.asarray(x, dtype=np.float32)
    context = np.asarray(context, dtype=np.float32)
    in_maps = prep_inputs(x, context, Wq, bq, Wk, bk, Wv, bv)
    if _CACHED_NC is None:
        _CACHED_NC = build_nc()
    nc = _CACHED_NC
    core_ids = list(range(B))
    res = run_bass_kernel_spmd(nc, in_maps, core_ids)
    return np.stack([res.results[i]["out"] for i in core_ids]).astype(np.float32)


# revision 15
# speedup vs baseline: 1.0061x; 1.0054x over previous
"""Cross-attention kernel for Trainium2, data-parallel over batch on 8 NeuronCores.

Per core (batch element b):
  q = x[b] @ Wq.T + bq ; k = c[b] @ Wk.T + bk ; v = c[b] @ Wv.T + bv
  out[b] = softmax(q @ k.T / sqrt(D)) @ v

Algebraic fold (host side): scores = q k^T = x (Wq^T Wk) c^T + rank-1 terms.
With M = Wq^T Wk and t = x M + bq Wk, scores = t c^T + f(s) where the f(s)
term is constant along the key axis and cancels in softmax exactly. This
removes the K projection entirely (256 of 1792 matmuls) and lets phase 2
stream c^T straight from SBUF as the key operand.

Device layout (all matmul operands bf16, fp32 accumulation):
  phase 1: TT[d',s] = (M^T-free: stationary M[d,d'] tiles) x^T, scaled 1/sqrt(D)
           V[t,e]  = c @ Wv.T + bv                                (t on partitions)
  phase 2: per 128-row tile of s: S = TT.T @ CT (psum), P = exp(S) + row sums
           via ACT, P transposed 128x128 via DMA xbar, O = P @ V, scaled by
           1/rowsum on drain.
"""

import numpy as np
import ml_dtypes

import concourse.bass as bass
import concourse.mybir as mybir
import concourse.tile as tile
from concourse import bacc
from concourse.bass_utils import run_bass_kernel_spmd

DIM = 1024
SEQ = 2048
B = 8
P = 128
DT = DIM // P        # 8 contraction tiles of 128
ST = SEQ // P        # 16 seq tiles of 128
KC = SEQ // 512      # 4 key chunks of 512
EC = DIM // 512      # 2 embed chunks of 512
F32 = mybir.dt.float32
BF16 = mybir.dt.bfloat16

_CACHED_NC = None


def build_nc():
    nc = bacc.Bacc(None, target_bir_lowering=False)

    xt = nc.declare_dram_parameter("xt", [DIM, SEQ], BF16, isOutput=False)
    ct = nc.declare_dram_parameter("ct", [DIM, SEQ], BF16, isOutput=False)
    # m pre-tiled on host: m[et, p, dt, j] = M[dt*128+p, et*128+j], so each
    # et tile lands as one DMA with 2 KB-contiguous per-partition rows.
    m = nc.declare_dram_parameter("m", [DT, P, DT, P], BF16, isOutput=False)
    # wv pre-tiled on host: wv[p, dt, e] = Wv[e, dt*128+p] (8 KB/partition).
    wvt = nc.declare_dram_parameter("wvt", [P, DT, DIM], BF16, isOutput=False)
    bts = nc.declare_dram_parameter("bts", [DT, P], F32, isOutput=False)
    bvb = nc.declare_dram_parameter("bvb", [P, DIM], F32, isOutput=False)
    out = nc.declare_dram_parameter("out", [SEQ, DIM], F32, isOutput=True)

    xt_r = xt.rearrange("(t p) s -> p t s", p=P)
    ct_r = ct.rearrange("(t p) s -> p t s", p=P)
    m_r = m.rearrange("e p t j -> p e t j")
    out_r = out.rearrange("(t p) e -> p t e", p=P)

    with tile.TileContext(nc) as tc:
        with (
            tc.tile_pool(name="resid", bufs=1) as resid,
            tc.tile_pool(name="singles", bufs=1) as singles,
        ):
            tt_sb = resid.tile([P, DT, SEQ], BF16, tag="tt")
            ct_sb = resid.tile([P, DT, SEQ], BF16, tag="ct")
            v_sb = resid.tile([P, ST, DIM], BF16, tag="v")

            bt_sb = singles.tile([P, DT], F32, tag="bt")
            bv_sb = singles.tile([P, DIM], F32, tag="bv")

            # ---------------- phase 1: projections ----------------
            with (
                tc.tile_pool(name="acts", bufs=1) as acts,
                tc.tile_pool(name="mpool", bufs=1) as mpool,
                tc.tile_pool(name="wvpool", bufs=1) as wvpool,
                tc.tile_pool(name="warmps", bufs=1, space="PSUM") as warmps,
                tc.tile_pool(name="ppool", bufs=6, space="PSUM") as ppool,
            ):
                # Dummy matmuls on a zeroed tile keep the PE busy through the
                # input-DMA window: HAM un-throttles before the real matmuls
                # start, instead of ramping on them.
                wsrc = acts.tile([P, 512], BF16, tag="warm")
                nc.gpsimd.memset(wsrc, 0.0)
                wps = warmps.tile([P, 512], F32, tag="wps")
                for i in range(26):
                    nc.tensor.matmul(
                        wps, wsrc[:, 0:P], wsrc, start=(i == 0), stop=(i == 25)
                    )
                xt_sb = acts.tile([P, DT, SEQ], BF16, tag="xt")
                # et-major so each et tile is one contiguous 2 KB/partition DMA
                m_sb = mpool.tile([P, DT, DT, P], BF16, tag="m")
                wv_t = wvpool.tile([P, DT, DIM], BF16, tag="wv")
                # Input DMA schedule, interleaved across the two HWDGE engines
                # (cheap descriptor issue) in consumption order, with gpsimd's
                # SWDGE path (~1us descriptor generation per dma_start)
                # carrying a few big batched transfers needed later.
                # Per-ring DMA throughput is descriptor-count-bound
                # (~18ns/desc), so every transfer here uses >=2KB
                # per-partition descriptors: xt in 1024-col half-row chunks
                # (the sc0+sc1 passes share half 0), m0/m1 as 2KB-desc tiles
                # on the HWDGE rings, and everything else as big batched
                # gpsimd transfers with 4-16KB descriptors.
                def xt_load(eng, h, dts):
                    for dt in dts:
                        eng.dma_start(
                            out=xt_sb[:, dt, h * 1024 : (h + 1) * 1024],
                            in_=xt_r[:, dt, h * 1024 : (h + 1) * 1024],
                        )

                def m_load(eng, et):
                    eng.dma_start(out=m_sb[:, et], in_=m_r[:, et])

                m_load(nc.sync, 0)
                m_load(nc.scalar, 1)
                xt_load(nc.sync, 0, (0, 1, 2))
                xt_load(nc.scalar, 0, (4, 5, 6))
                xt_load(nc.gpsimd, 0, (3, 7))
                for et in range(2, DT):
                    m_load(nc.sync if et % 2 == 0 else nc.scalar, et)
                xt_load(nc.sync, 1, range(0, 4))
                xt_load(nc.scalar, 1, range(4, 8))
                nc.gpsimd.dma_start(out=bt_sb, in_=bts.rearrange("t p -> p t"))
                nc.gpsimd.dma_start(out=bv_sb, in_=bvb[:, :])
                nc.gpsimd.dma_start(out=ct_sb[:, :], in_=ct_r[:, :, :])
                nc.gpsimd.dma_start(out=wv_t[:, :, :], in_=wvt[:, :, :])

                # t projection: out[d'128, s512] accumulated over d.
                # First two sc passes (both inside xt half 0) are interleaved
                # per et so each m tile is consumed at the rate the DMA rings
                # deliver them; the sc2/sc3 passes then run with everything
                # resident.
                def t_group(sc, et):
                    ps = ppool.tile([P, 512], F32, tag="proj")
                    for dt in range(DT):
                        nc.tensor.matmul(
                            ps,
                            m_sb[:, et, dt],
                            xt_sb[:, dt, sc * 512 : (sc + 1) * 512],
                            start=(dt == 0),
                            stop=(dt == DT - 1),
                        )
                    nc.scalar.activation(
                        out=tt_sb[:, et, sc * 512 : (sc + 1) * 512],
                        in_=ps,
                        func=mybir.ActivationFunctionType.Identity,
                        bias=bt_sb[:, et : et + 1],
                        scale=1.0 / 32.0,
                    )

                for et in range(DT):
                    t_group(0, et)
                    t_group(1, et)
                for sc in (2, 3):
                    for et in range(DT):
                        t_group(sc, et)

                # v projection: out[t128, e512], CT tiles stationary
                for tt in range(ST):
                    for ec in range(EC):
                        ps = ppool.tile([P, 512], F32, tag="proj")
                        for dt in range(DT):
                            nc.tensor.matmul(
                                ps,
                                ct_sb[:, dt, tt * P : (tt + 1) * P],
                                wv_t[:, dt, ec * 512 : (ec + 1) * 512],
                                start=(dt == 0),
                                stop=(dt == DT - 1),
                            )
                        nc.vector.tensor_add(
                            out=v_sb[:, tt, ec * 512 : (ec + 1) * 512],
                            in0=ps,
                            in1=bv_sb[:, ec * 512 : (ec + 1) * 512],
                        )

            # ---------------- phase 2: attention ----------------
            # Software-pipelined: S/exp/transpose for tile st is emitted
            # before O/store for tile st-1, so the PE chews on S(st) while
            # the xbar transpose of P(st-1) completes.
            with (
                tc.tile_pool(name="attn", bufs=3) as attn,
                tc.tile_pool(name="stats", bufs=4) as stats,
                tc.tile_pool(name="spsum", bufs=5, space="PSUM") as spsum,
                tc.tile_pool(name="opsum", bufs=3, space="PSUM") as opsum,
            ):
                def emit_s_stage(st):
                    p_sb = attn.tile([P, SEQ], BF16, tag="p")
                    sums = stats.tile([P, KC], F32, tag="sums")
                    for kc in range(KC):
                        sp = spsum.tile([P, 512], F32, tag="s")
                        for dt in range(DT):
                            nc.tensor.matmul(
                                sp,
                                tt_sb[:, dt, st * P : (st + 1) * P],
                                ct_sb[:, dt, kc * 512 : (kc + 1) * 512],
                                start=(dt == 0),
                                stop=(dt == DT - 1),
                            )
                        nc.scalar.activation(
                            out=p_sb[:, kc * 512 : (kc + 1) * 512],
                            in_=sp,
                            func=mybir.ActivationFunctionType.Exp,
                            accum_out=sums[:, kc : kc + 1],
                        )
                    ssum = stats.tile([P, 1], F32, tag="ssum")
                    rsum = stats.tile([P, 1], F32, tag="rsum")
                    nc.vector.reduce_sum(out=ssum, in_=sums, axis=mybir.AxisListType.X)
                    nc.vector.reciprocal(out=rsum, in_=ssum)

                    # One xbar transpose for the whole row block:
                    # pt[p, tt, f] = p_sb[f, tt*128 + p]
                    pt_sb = attn.tile([P, ST, P], BF16, tag="pt")
                    nc.sync.dma_start_transpose(out=pt_sb, in_=p_sb[:, :])
                    return pt_sb, rsum

                def emit_o_stage(st, pt_sb, rsum):
                    for ec in range(EC):
                        op = opsum.tile([P, 512], F32, tag="o")
                        for tt in range(ST):
                            nc.tensor.matmul(
                                op,
                                pt_sb[:, tt],
                                v_sb[:, tt, ec * 512 : (ec + 1) * 512],
                                start=(tt == 0),
                                stop=(tt == ST - 1),
                            )
                        # Final drain of the whole kernel is on the critical
                        # path: split it so only a 256-col scale+store chain
                        # trails the last matmul.
                        last = st == ST - 1 and ec == EC - 1
                        for (lo, hi) in ((0, 256), (256, 512)) if last else ((0, 512),):
                            o_sb = attn.tile([P, hi - lo], F32, tag=f"o{hi - lo}")
                            nc.vector.tensor_scalar_mul(
                                out=o_sb, in0=op[:, lo:hi], scalar1=rsum
                            )
                            nc.scalar.dma_start(
                                out=out_r[:, st, ec * 512 + lo : ec * 512 + hi],
                                in_=o_sb,
                            )

                pending = None
                for st in range(ST):
                    cur = emit_s_stage(st)
                    if pending is not None:
                        emit_o_stage(st - 1, *pending)
                    pending = cur
                emit_o_stage(ST - 1, *pending)

    nc.compile()
    return nc


def prep_inputs(x, context, Wq, bq, Wk, bk, Wv, bv):
    """Host-side prep: per-batch transposed bf16 activations, the folded
    score matrix M = Wq^T Wk (bf16), transposed bf16 V weight, tiled fp32
    biases. Returns per-core input maps."""
    bf = ml_dtypes.bfloat16
    Wq = np.asarray(Wq, dtype=np.float32)
    Wk = np.asarray(Wk, dtype=np.float32)
    mfull = Wq.T @ Wk
    # m[et, p, dt, j] = M[dt*128+p, et*128+j]
    m = np.ascontiguousarray(
        mfull.reshape(DT, P, DT, P).transpose(2, 1, 0, 3)
    ).astype(bf)
    # wvt[p, dt, e] = Wv.T[dt*128+p, e]
    wvt = np.ascontiguousarray(
        np.asarray(Wv, dtype=np.float32).T.reshape(DT, P, DIM).transpose(1, 0, 2)
    ).astype(bf)
    # t = x M + bq Wk; the q.bk rank-1 term is constant along keys and
    # cancels in softmax. 1/sqrt(D) folded into the t projection.
    bt = (np.asarray(bq, dtype=np.float32) @ Wk) / 32.0
    bts = bt.reshape(DT, P)
    bvb = np.ascontiguousarray(
        np.broadcast_to(np.asarray(bv, dtype=np.float32), (P, DIM))
    )
    in_maps = []
    for b in range(B):
        in_maps.append(
            {
                "xt": np.ascontiguousarray(x[b].T).astype(bf),
                "ct": np.ascontiguousarray(context[b].T).astype(bf),
                "m": m,
                "wvt": wvt,
                "bts": bts,
                "bvb": bvb,
            }
        )
    return in_maps


def kernel(x, context, Wq, bq, Wk, bk, Wv, bv):
    global _CACHED_NC
    x = np.asarray(x, dtype=np.float32)
    context = np.asarray(context, dtype=np.float32)
    in_maps = prep_inputs(x, context, Wq, bq, Wk, bk, Wv, bv)
    if _CACHED_NC is None:
        _CACHED_NC = build_nc()
    nc = _CACHED_NC
    core_ids = list(range(B))
    res = run_bass_kernel_spmd(nc, in_maps, core_ids)
    return np.stack([res.results[i]["out"] for i in core_ids]).astype(np.float32)


# revision 16
# speedup vs baseline: 1.0131x; 1.0070x over previous
"""Cross-attention kernel for Trainium2, data-parallel over batch on 8 NeuronCores.

Per core (batch element b):
  q = x[b] @ Wq.T + bq ; k = c[b] @ Wk.T + bk ; v = c[b] @ Wv.T + bv
  out[b] = softmax(q @ k.T / sqrt(D)) @ v

Algebraic fold (host side): scores = q k^T = x (Wq^T Wk) c^T + rank-1 terms.
With M = Wq^T Wk and t = x M + bq Wk, scores = t c^T + f(s) where the f(s)
term is constant along the key axis and cancels in softmax exactly. This
removes the K projection entirely (256 of 1792 matmuls) and lets phase 2
stream c^T straight from SBUF as the key operand.

Device layout (all matmul operands bf16, fp32 accumulation):
  phase 1: TT[d',s] = (M^T-free: stationary M[d,d'] tiles) x^T, scaled 1/sqrt(D)
           V[t,e]  = c @ Wv.T + bv                                (t on partitions)
  phase 2: per 128-row tile of s: S = TT.T @ CT (psum), P = exp(S) + row sums
           via ACT, P transposed 128x128 via DMA xbar, O = P @ V, scaled by
           1/rowsum on drain.
"""

import numpy as np
import ml_dtypes

import concourse.bass as bass
import concourse.mybir as mybir
import concourse.tile as tile
from concourse import bacc
from concourse.bass_utils import run_bass_kernel_spmd

DIM = 1024
SEQ = 2048
B = 8
P = 128
DT = DIM // P        # 8 contraction tiles of 128
ST = SEQ // P        # 16 seq tiles of 128
KC = SEQ // 512      # 4 key chunks of 512
EC = DIM // 512      # 2 embed chunks of 512
F32 = mybir.dt.float32
BF16 = mybir.dt.bfloat16

_CACHED_NC = None


def build_nc():
    nc = bacc.Bacc(None, target_bir_lowering=False)

    xt = nc.declare_dram_parameter("xt", [DIM, SEQ], BF16, isOutput=False)
    ct = nc.declare_dram_parameter("ct", [DIM, SEQ], BF16, isOutput=False)
    # m pre-tiled on host: m[et, p, dt, j] = M[dt*128+p, et*128+j], so each
    # et tile lands as one DMA with 2 KB-contiguous per-partition rows.
    m = nc.declare_dram_parameter("m", [DT, P, DT, P], BF16, isOutput=False)
    # wv pre-tiled on host: wv[p, dt, e] = Wv[e, dt*128+p] (8 KB/partition).
    wvt = nc.declare_dram_parameter("wvt", [P, DT, DIM], BF16, isOutput=False)
    bts = nc.declare_dram_parameter("bts", [DT, P], F32, isOutput=False)
    bvb = nc.declare_dram_parameter("bvb", [P, DIM], F32, isOutput=False)
    out = nc.declare_dram_parameter("out", [SEQ, DIM], F32, isOutput=True)

    xt_r = xt.rearrange("(t p) s -> p t s", p=P)
    ct_r = ct.rearrange("(t p) s -> p t s", p=P)
    m_r = m.rearrange("e p t j -> p e t j")
    out_r = out.rearrange("(t p) e -> p t e", p=P)

    with tile.TileContext(nc) as tc:
        with (
            tc.tile_pool(name="resid", bufs=1) as resid,
            tc.tile_pool(name="singles", bufs=1) as singles,
        ):
            tt_sb = resid.tile([P, DT, SEQ], BF16, tag="tt")
            ct_sb = resid.tile([P, DT, SEQ], BF16, tag="ct")
            v_sb = resid.tile([P, ST, DIM], BF16, tag="v")

            bt_sb = singles.tile([P, DT], F32, tag="bt")
            bv_sb = singles.tile([P, DIM], F32, tag="bv")

            # ---------------- phase 1: projections ----------------
            with (
                tc.tile_pool(name="acts", bufs=1) as acts,
                tc.tile_pool(name="mpool", bufs=1) as mpool,
                tc.tile_pool(name="wvpool", bufs=1) as wvpool,
                tc.tile_pool(name="warmps", bufs=1, space="PSUM") as warmps,
                tc.tile_pool(name="ppool", bufs=6, space="PSUM") as ppool,
            ):
                # Dummy matmuls on a zeroed tile keep the PE busy through the
                # input-DMA window: HAM un-throttles before the real matmuls
                # start, instead of ramping on them.
                wsrc = acts.tile([P, 512], BF16, tag="warm")
                nc.gpsimd.memset(wsrc, 0.0)
                wps = warmps.tile([P, 512], F32, tag="wps")
                for i in range(26):
                    nc.tensor.matmul(
                        wps, wsrc[:, 0:P], wsrc, start=(i == 0), stop=(i == 25)
                    )
                xt_sb = acts.tile([P, DT, SEQ], BF16, tag="xt")
                # et-major so each et tile is one contiguous 2 KB/partition DMA
                m_sb = mpool.tile([P, DT, DT, P], BF16, tag="m")
                wv_t = wvpool.tile([P, DT, DIM], BF16, tag="wv")
                # Input DMA schedule, interleaved across the two HWDGE engines
                # (cheap descriptor issue) in consumption order, with gpsimd's
                # SWDGE path (~1us descriptor generation per dma_start)
                # carrying a few big batched transfers needed later.
                # Per-ring DMA throughput is descriptor-count-bound
                # (~18ns/desc), so every transfer here uses >=2KB
                # per-partition descriptors: xt in 1024-col half-row chunks
                # (the sc0+sc1 passes share half 0), m0/m1 as 2KB-desc tiles
                # on the HWDGE rings, and everything else as big batched
                # gpsimd transfers with 4-16KB descriptors.
                def xt_load(eng, h, dts):
                    for dt in dts:
                        eng.dma_start(
                            out=xt_sb[:, dt, h * 1024 : (h + 1) * 1024],
                            in_=xt_r[:, dt, h * 1024 : (h + 1) * 1024],
                        )

                def m_load(eng, et):
                    eng.dma_start(out=m_sb[:, et], in_=m_r[:, et])

                m_load(nc.sync, 0)
                m_load(nc.scalar, 1)
                xt_load(nc.sync, 0, (0, 1))
                xt_load(nc.scalar, 0, (4, 5))
                xt_load(nc.gpsimd, 0, (2, 3, 6, 7))
                for et in range(2, DT):
                    m_load(nc.sync if et % 2 == 0 else nc.scalar, et)
                xt_load(nc.sync, 1, range(0, 4))
                xt_load(nc.scalar, 1, range(4, 8))
                nc.gpsimd.dma_start(out=bt_sb, in_=bts.rearrange("t p -> p t"))
                nc.gpsimd.dma_start(out=bv_sb, in_=bvb[:, :])
                nc.gpsimd.dma_start(out=ct_sb[:, :], in_=ct_r[:, :, :])
                nc.gpsimd.dma_start(out=wv_t[:, :, :], in_=wvt[:, :, :])

                # t projection: out[d'128, s512] accumulated over d.
                # First two sc passes (both inside xt half 0) are interleaved
                # per et so each m tile is consumed at the rate the DMA rings
                # deliver them; the sc2/sc3 passes then run with everything
                # resident.
                def t_group(sc, et):
                    ps = ppool.tile([P, 512], F32, tag="proj")
                    for dt in range(DT):
                        nc.tensor.matmul(
                            ps,
                            m_sb[:, et, dt],
                            xt_sb[:, dt, sc * 512 : (sc + 1) * 512],
                            start=(dt == 0),
                            stop=(dt == DT - 1),
                        )
                    nc.scalar.activation(
                        out=tt_sb[:, et, sc * 512 : (sc + 1) * 512],
                        in_=ps,
                        func=mybir.ActivationFunctionType.Identity,
                        bias=bt_sb[:, et : et + 1],
                        scale=1.0 / 32.0,
                    )

                for et in range(DT):
                    t_group(0, et)
                    t_group(1, et)
                for sc in (2, 3):
                    for et in range(DT):
                        t_group(sc, et)

                # v projection: out[t128, e512], CT tiles stationary
                for tt in range(ST):
                    for ec in range(EC):
                        ps = ppool.tile([P, 512], F32, tag="proj")
                        for dt in range(DT):
                            nc.tensor.matmul(
                                ps,
                                ct_sb[:, dt, tt * P : (tt + 1) * P],
                                wv_t[:, dt, ec * 512 : (ec + 1) * 512],
                                start=(dt == 0),
                                stop=(dt == DT - 1),
                            )
                        nc.vector.tensor_add(
                            out=v_sb[:, tt, ec * 512 : (ec + 1) * 512],
                            in0=ps,
                            in1=bv_sb[:, ec * 512 : (ec + 1) * 512],
                        )

            # ---------------- phase 2: attention ----------------
            # Software-pipelined: S/exp/transpose for tile st is emitted
            # before O/store for tile st-1, so the PE chews on S(st) while
            # the xbar transpose of P(st-1) completes.
            with (
                tc.tile_pool(name="attn", bufs=3) as attn,
                tc.tile_pool(name="stats", bufs=4) as stats,
                tc.tile_pool(name="spsum", bufs=5, space="PSUM") as spsum,
                tc.tile_pool(name="opsum", bufs=3, space="PSUM") as opsum,
            ):
                def emit_s_stage(st):
                    p_sb = attn.tile([P, SEQ], BF16, tag="p")
                    sums = stats.tile([P, KC], F32, tag="sums")
                    for kc in range(KC):
                        sp = spsum.tile([P, 512], F32, tag="s")
                        for dt in range(DT):
                            nc.tensor.matmul(
                                sp,
                                tt_sb[:, dt, st * P : (st + 1) * P],
                                ct_sb[:, dt, kc * 512 : (kc + 1) * 512],
                                start=(dt == 0),
                                stop=(dt == DT - 1),
                            )
                        nc.scalar.activation(
                            out=p_sb[:, kc * 512 : (kc + 1) * 512],
                            in_=sp,
                            func=mybir.ActivationFunctionType.Exp,
                            accum_out=sums[:, kc : kc + 1],
                        )
                    ssum = stats.tile([P, 1], F32, tag="ssum")
                    rsum = stats.tile([P, 1], F32, tag="rsum")
                    nc.vector.reduce_sum(out=ssum, in_=sums, axis=mybir.AxisListType.X)
                    nc.vector.reciprocal(out=rsum, in_=ssum)

                    # One xbar transpose for the whole row block:
                    # pt[p, tt, f] = p_sb[f, tt*128 + p]
                    pt_sb = attn.tile([P, ST, P], BF16, tag="pt")
                    nc.sync.dma_start_transpose(out=pt_sb, in_=p_sb[:, :])
                    return pt_sb, rsum

                def emit_o_stage(st, pt_sb, rsum):
                    for ec in range(EC):
                        op = opsum.tile([P, 512], F32, tag="o")
                        for tt in range(ST):
                            nc.tensor.matmul(
                                op,
                                pt_sb[:, tt],
                                v_sb[:, tt, ec * 512 : (ec + 1) * 512],
                                start=(tt == 0),
                                stop=(tt == ST - 1),
                            )
                        # Final drain of the whole kernel is on the critical
                        # path: split it so only a 256-col scale+store chain
                        # trails the last matmul.
                        last = st == ST - 1 and ec == EC - 1
                        for (lo, hi) in ((0, 256), (256, 512)) if last else ((0, 512),):
                            o_sb = attn.tile([P, hi - lo], F32, tag=f"o{hi - lo}")
                            nc.vector.tensor_scalar_mul(
                                out=o_sb, in0=op[:, lo:hi], scalar1=rsum
                            )
                            nc.scalar.dma_start(
                                out=out_r[:, st, ec * 512 + lo : ec * 512 + hi],
                                in_=o_sb,
                            )

                pending = None
                for st in range(ST):
                    cur = emit_s_stage(st)
                    if pending is not None:
                        emit_o_stage(st - 1, *pending)
                    pending = cur
                emit_o_stage(ST - 1, *pending)

    nc.compile()
    return nc


def prep_inputs(x, context, Wq, bq, Wk, bk, Wv, bv):
    """Host-side prep: per-batch transposed bf16 activations, the folded
    score matrix M = Wq^T Wk (bf16), transposed bf16 V weight, tiled fp32
    biases. Returns per-core input maps."""
    bf = ml_dtypes.bfloat16
    Wq = np.asarray(Wq, dtype=np.float32)
    Wk = np.asarray(Wk, dtype=np.float32)
    mfull = Wq.T @ Wk
    # m[et, p, dt, j] = M[dt*128+p, et*128+j]
    m = np.ascontiguousarray(
        mfull.reshape(DT, P, DT, P).transpose(2, 1, 0, 3)
    ).astype(bf)
    # wvt[p, dt, e] = Wv.T[dt*128+p, e]
    wvt = np.ascontiguousarray(
        np.asarray(Wv, dtype=np.float32).T.reshape(DT, P, DIM).transpose(1, 0, 2)
    ).astype(bf)
    # t = x M + bq Wk; the q.bk rank-1 term is constant along keys and
    # cancels in softmax. 1/sqrt(D) folded into the t projection.
    bt = (np.asarray(bq, dtype=np.float32) @ Wk) / 32.0
    bts = bt.reshape(DT, P)
    bvb = np.ascontiguousarray(
        np.broadcast_to(np.asarray(bv, dtype=np.float32), (P, DIM))
    )
    in_maps = []
    for b in range(B):
        in_maps.append(
            {
                "xt": np.ascontiguousarray(x[b].T).astype(bf),
                "ct": np.ascontiguousarray(context[b].T).astype(bf),
                "m": m,
                "wvt": wvt,
                "bts": bts,
                "bvb": bvb,
            }
        )
    return in_maps


def kernel(x, context, Wq, bq, Wk, bk, Wv, bv):
    global _CACHED_NC
    x = np.asarray(x, dtype=np.float32)
    context = np.asarray(context, dtype=np.float32)
    in_maps = prep_inputs(x, context, Wq, bq, Wk, bk, Wv, bv)
    if _CACHED_NC is None:
        _CACHED_NC = build_nc()
    nc = _CACHED_NC
    core_ids = list(range(B))
    res = run_bass_kernel_spmd(nc, in_maps, core_ids)
    return np.stack([res.results[i]["out"] for i in core_ids]).astype(np.float32)
